# revision 2
# baseline (speedup 1.0000x reference)
"""Trainium2 Bass kernel for nn_Decoder (LSTM, B=128 T=512 H=1024 O=128).

Single-core (collective floors ~10us/step kill cross-core sharding; all
per-step engine work scales with free dims, so batch sharding saves nothing).

fp8e4 DoubleRow recurrent matmul (K-pairs, ~1.77x PE ALU vs bf16), stationary
h-pair reused across column-blocks per K-sweep (kp-outer) so LDWEIGHTS
hides. Per-step seed gates = x_t*w_ih + b via scalar_tensor_tensor on
GPSIMD/DVE directly into PSUM (start=False matmuls accumulate on top).
Gate columns host-permuted per 128-chunk as [i|f|o|g]; eltwise per pair of
chunks: ACT sigmoid/tanh -> bf16, GPSIMD t1=sf*c, DVE t2/c/hbf, PE transposes
into a shared PSUM tile, DVE copy converts bf16->fp8 pairs for the next step.
Last K_CLEAN=16 steps run in bf16 to wash out fp8 noise (rel err ~7e-3).
"""

import os
import sys

sys.path.insert(0, "/opt/trn_rl_repo")
os.environ.setdefault("JAX_PLATFORMS", "")

from contextlib import ExitStack

import numpy as np
import ml_dtypes

import concourse.bass as bass
import concourse.mybir as mybir
import concourse.tile as tile
from concourse.bass import ds

B, T, H, O = 128, 512, 1024, 128
KC = H // 128           # 8 k-subtiles / h-chunks
NP = KC // 2            # 4 k-pairs / chunk-pairs
GW = 512                # gate columns per H-chunk: [i|f|o|g] x 128
BF16 = mybir.dt.bfloat16
F32 = mybir.dt.float32
FP8 = mybir.dt.float8e4
DR = mybir.MatmulPerfMode.DoubleRow
MULT = mybir.AluOpType.mult
ADD = mybir.AluOpType.add

K_CLEAN = int(os.environ.get("KERNEL_K_CLEAN", "16"))  # bf16 tail steps
K_F8TAIL = 16          # fp8 steps unrolled outside the loop (phase switch)
_UNROLL = int(os.environ.get("KERNEL_UNROLL", "16"))


# ---------------------------------------------------------------- drain patch
# walrus codegen limit: InstDrain on the SP engine accepts a single sync-wait
# command; TileContext's exit drain aggregates one wait per outstanding
# logical processor onto one drain. Split them across a chain of drains.
def _apply_drain_patch():
    import concourse.tile as _tile
    from concourse.vector_clock import ScopedClock as _ScopedClock

    if getattr(_tile.TileContext, "_drain_patch_applied", False):
        return

    def _patched(self, tick_clock, wait_clock):
        drain_inst = self.nc.sync.drain()
        wait_clock.add_sem_waits(
            drain_inst.ins, _ScopedClock({None: tick_clock.global_clock})
        )
        si = drain_inst.ins.sync_info
        waits = list(si.on_wait) if si is not None and si.on_wait else []
        if len(waits) > 1:
            si.on_wait = waits[:1]
            for w in waits[1:]:
                extra = self.nc.sync.drain()
                extra.ins.sync_info = mybir.SyncInfo(on_wait=[w], on_update=[])
        self.nc.all_engine_barrier()
        assert self.sems is not None
        popped = self.nc._tile_sem_poison_stack.pop()
        assert popped is self._sem_poison
        self.nc.clear_and_free_semaphores(list(self.sems.allocated().values()))
        self.nc.all_engine_barrier()

    _tile.TileContext._drain_and_barrier = _patched
    _tile.TileContext._drain_patch_applied = True


# ----------------------------------------------------- wait-splitting post-pass
# This walrus build accepts at most 1 sync-wait on engine instructions; Tile
# attaches up to ~4. Split the excess onto InstNoOp carriers on the same engine.
def _split_excess_waits(nc):
    n_added = 0
    for f in nc.m.functions:
        for bb in f.blocks:
            insts = bb.instructions
            out = []
            changed = False
            for inst in insts:
                si = inst.sync_info
                waits = list(si.on_wait) if si is not None and si.on_wait else []
                lim = 1
                if len(waits) > lim:
                    keep = waits[len(waits) - lim:]
                    rest = waits[: len(waits) - lim]
                    while rest:
                        chunk, rest = rest[:1], rest[1:]
                        nop = mybir.InstNoOp(
                            name=f"waitnop-{n_added}", ins=[], outs=[]
                        )
                        nop.engine = inst.engine
                        nop.sync_info = mybir.SyncInfo(on_wait=chunk, on_update=[])
                        out.append(nop)
                        n_added += 1
                    si.on_wait = keep
                    changed = True
                out.append(inst)
            if changed:
                bb.instructions = out
    return n_added


# ------------------------------------------------------------- program build
def build_program(t_loop=T - K_CLEAN - K_F8TAIL, t_f8tail=K_F8TAIL,
                  t_bf16=K_CLEAN, unroll=_UNROLL, split_waits=True,
                  seed_eng=None):
    """t_loop fp8 steps in a For_i (must be unroll-divisible), then t_f8tail
    fp8 steps unrolled (last one writes bf16 state), then t_bf16 bf16 steps."""
    _apply_drain_patch()
    assert t_loop % unroll == 0 and unroll % 2 == 0
    assert t_f8tail % 2 == 0 and t_f8tail > 0 and t_bf16 > 0
    nc = bass.Bass("TRN2", debug=False)

    w8_d = nc.dram_tensor("w8", (128, NP, 2, 4 * H), FP8, kind="ExternalInput").ap()
    wbf_d = nc.dram_tensor("wbf", (128, KC, 4 * H), BF16, kind="ExternalInput").ap()
    wb_d = nc.dram_tensor("wb", (128, 4 * H), BF16, kind="ExternalInput").ap()
    bb_d = nc.dram_tensor("bb", (128, 4 * H), F32, kind="ExternalInput").ap()
    xct_d = nc.dram_tensor("xct", (B, T), F32, kind="ExternalInput").ap()
    xaug_d = nc.dram_tensor("xaug", (2, B, T), BF16, kind="ExternalInput").ap()
    wba_d = nc.dram_tensor("wba", (2, 4 * H), BF16, kind="ExternalInput").ap()
    h80_d = nc.dram_tensor("h80", (128, NP, 2, B), FP8, kind="ExternalInput").ap()
    c0_d = nc.dram_tensor("c0", (B, H), F32, kind="ExternalInput").ap()
    fcw_d = nc.dram_tensor("fcw", (128, H), BF16, kind="ExternalInput").ap()
    fca_d = nc.dram_tensor("fca", (2, 128), BF16, kind="ExternalInput").ap()
    id_d = nc.dram_tensor("ident", (128, 128), BF16, kind="ExternalInput").ap()
    out_d = nc.dram_tensor("out", (B, O), F32, kind="ExternalOutput").ap()

    if seed_eng is None:
        seed_eng = os.environ.get("SEED_ENG", "gg vv")  # per-pair engines
    seed_eng = seed_eng.split() if isinstance(seed_eng, str) else seed_eng
    # seed_eng like ["gg", "vv"]: pair0,1 engines then pair2,3 (g/v/p = gpsimd/DVE/PE)

    with tile.TileContext(nc) as tc:
        with ExitStack() as ctx:
            consts = ctx.enter_context(tc.tile_pool(name="consts", bufs=1))
            state = ctx.enter_context(tc.tile_pool(name="state", bufs=1))
            work = ctx.enter_context(
                tc.tile_pool(name="work", bufs=int(os.environ.get("WORK_BUFS", "4")))
            )
            xap = ctx.enter_context(tc.tile_pool(name="xap", bufs=8))
            psum = ctx.enter_context(tc.tile_pool(name="psum", bufs=3, space="PSUM"))
            ptp = ctx.enter_context(tc.tile_pool(name="ptp", bufs=2, space="PSUM"))

            # resident weights
            w8 = consts.tile([128, NP, 2, 4 * H], FP8, tag="w8")
            nc.gpsimd.dma_start(out=w8, in_=w8_d)
            wbf = consts.tile([128, KC, 4 * H], BF16, tag="wbf")
            nc.gpsimd.dma_start(out=wbf, in_=wbf_d)
            wb = consts.tile([128, 4 * H], BF16, tag="wb")
            nc.gpsimd.dma_start(out=wb, in_=wb_d)
            wba = consts.tile([2, 4 * H], BF16, tag="wba")
            nc.gpsimd.dma_start(out=wba, in_=wba_d)
            bb = consts.tile([128, 4 * H], F32, tag="bb")
            nc.gpsimd.dma_start(out=bb, in_=bb_d)
            fcw = consts.tile([128, H], BF16, tag="fcw")
            nc.gpsimd.dma_start(out=fcw, in_=fcw_d)
            ident = consts.tile([128, 128], BF16, tag="ident")
            nc.gpsimd.dma_start(out=ident, in_=id_d)
            fcb_t = consts.tile([1, 128], BF16, tag="fcb_t")
            nc.gpsimd.dma_start(out=fcb_t, in_=fca_d[0:1, :])
            ones_t = consts.tile([1, 128], BF16, tag="ones_t")
            nc.gpsimd.dma_start(out=ones_t, in_=fca_d[1:2, :])

            # state
            h8_a = [state.tile([128, 2, B], FP8, tag=f"h8a{k}", name=f"h8a{k}")
                    for k in range(NP)]
            h8_b = [state.tile([128, 2, B], FP8, tag=f"h8b{k}", name=f"h8b{k}")
                    for k in range(NP)]
            hb_a = state.tile([128, KC, B], BF16, tag="hba")
            hb_b = state.tile([128, KC, B], BF16, tag="hbb")
            c_sb = state.tile([B, H], F32, tag="c")
            for k in range(NP):
                nc.gpsimd.dma_start(out=h8_a[k], in_=h80_d[:, k])
            nc.gpsimd.dma_start(out=c_sb, in_=c0_d)

            def step(iv_base, local_t, mode, cur, nxt, out_bf16=False,
                     prev_tail=None):
                """One LSTM step. mode: 'f8' (DoubleRow) or 'bf' (bf16 MMs).
                Returns a closure emitting pairs 2,3 transposes+copies; the
                caller passes it back as prev_tail of the NEXT step so those
                PE/DVE ops land after the next step's aug + kp0 MMs in queue
                order (they wait on this step's slow eltwise tail; emitting
                them inline would stall the in-order PE and drop its p-state
                clock ramp)."""
                xa3 = xap.tile([2, B, 1], BF16, tag="xaug", name="xaug")
                if isinstance(iv_base, int):
                    off = iv_base + local_t
                    nc.sync.dma_start(out=xa3, in_=xaug_d[:, :, off : off + 1])
                else:
                    off = iv_base + local_t
                    nc.sync.dma_start(out=xa3, in_=xaug_d[:, :, ds(off, 1)])
                xa = xa3[:, :, 0]

                ps_tiles = []
                for p in range(4):
                    ps = psum.tile([B, 2, GW], F32, tag="gates", name=f"ps{p}")
                    ps_tiles.append(ps)

                # seed one pair tile with x_t*w_ih + b via PE K=2 aug matmuls
                # (start=True resets PSUM; main MMs accumulate on top). Pair 3
                # shares a PSUM buffer with pair 0 (bufs=3, 4 pairs), so its
                # aug WARs on pair 0's sigmoid: emit each aug just before its
                # pair's first main MM, not all at the top, or the in-order PE
                # queue stalls ~5us/step on that WAR.
                def aug(p):
                    for h_ in range(2):
                        cc = 2 * p + h_
                        nc.tensor.matmul(
                            ps_tiles[p][:, h_, :], lhsT=xa,
                            rhs=wba[:, cc * GW : (cc + 1) * GW],
                            start=True, stop=False, skip_group_check=True,
                        )

                nk = NP if mode == "f8" else KC

                def mm(q, k):
                    for h_ in range(2):
                        sl = ps_tiles[q][:, h_, :]
                        cc = 2 * q + h_
                        cols = slice(cc * GW, (cc + 1) * GW)
                        if mode == "f8":
                            nc.tensor.matmul(
                                sl, lhsT=cur[k], rhs=w8[:, k, :, cols],
                                start=False, stop=(k == NP - 1),
                                perf_mode=DR, skip_group_check=True,
                            )
                        else:
                            nc.tensor.matmul(
                                sl, lhsT=cur[:, k], rhs=wbf[:, k, cols],
                                start=False, stop=(k == KC - 1),
                                skip_group_check=True,
                            )

                # half A: pairs 0,1 kp-outer; prev step's deferred tail lands
                # after the kp0 sweep (its h data is ready by then)
                aug(0)
                aug(1)
                mm(0, 0)
                mm(1, 0)
                if prev_tail is not None:
                    prev_tail()
                for k in range(1, nk):
                    mm(0, k)
                    mm(1, k)
                # half B: both augs grouped (one bf16 block, fewer PE mode
                # transitions); by halfB pair 0's sigmoid has nearly freed the
                # PSUM buffer shared with pair 3
                aug(2)
                aug(3)
                for k in range(nk):
                    mm(2, k)
                    mm(3, k)

                # eltwise per pair. Pairs 2,3 (latest to finish, on the next
                # step's critical path) run split per-chunk; their transposes
                # and PSUM->SBUF copies are deferred into the next step.
                tp_tiles = [
                    ptp.tile([128, 512], BF16, tag="tp", name=f"tp{g}")
                    for g in range(2)
                ]
                tails = []
                for p in range(4):
                    ps3 = ps_tiles[p]
                    tpx = tp_tiles[p // 2]
                    base = (p % 2) * 256
                    halves = ((0, 2),) if p < 2 else ((0, 1), (1, 2))
                    tc_args = []
                    for j0, j1 in halves:
                        nj = j1 - j0
                        jj = slice(j0, j1)
                        sig = work.tile([B, nj, 384], BF16, tag=f"sig{nj}",
                                        name="sig")
                        nc.scalar.activation(
                            sig, ps3[:, jj, 0:384],
                            mybir.ActivationFunctionType.Sigmoid,
                        )
                        tg = work.tile([B, nj, 128], BF16, tag=f"tg{nj}", name="tg")
                        nc.scalar.activation(
                            tg, ps3[:, jj, 384:512],
                            mybir.ActivationFunctionType.Tanh,
                        )
                        c3 = c_sb[
                            :, p * 256 + j0 * 128 : p * 256 + j1 * 128
                        ].rearrange("p (c x) -> p c x", c=nj)
                        t1 = work.tile([B, nj, 128], F32, tag=f"t1{nj}", name="t1")
                        nc.vector.tensor_mul(t1, sig[:, :, 128:256], c3)
                        t2 = work.tile([B, nj, 128], BF16, tag=f"t2{nj}", name="t2")
                        nc.vector.tensor_mul(t2, sig[:, :, 0:128], tg)
                        nc.vector.tensor_add(c3, t1, t2)
                        tanc = work.tile([B, nj, 128], BF16, tag=f"tanc{nj}",
                                         name="tanc")
                        nc.scalar.activation(
                            tanc, c3, mybir.ActivationFunctionType.Tanh
                        )
                        hbf = work.tile([B, nj, 128], BF16, tag=f"hbf{nj}",
                                        name="hbf")
                        nc.vector.tensor_mul(hbf, sig[:, :, 256:384], tanc)
                        tc_args.append((hbf, j0, j1))

                    def emit_tc(p=p, tpx=tpx, base=base, tc_args=tuple(tc_args)):
                        for hbf, j0, j1 in tc_args:
                            nj = j1 - j0
                            for j in range(nj):
                                nc.tensor.transpose(
                                    tpx[:, base + (j0 + j) * 128
                                        : base + (j0 + j + 1) * 128],
                                    hbf[:, j, :], ident,
                                )
                            tpv = tpx[
                                :, base + j0 * 128 : base + j1 * 128
                            ].rearrange("p (c x) -> p c x", c=nj)
                            if mode == "f8" and not out_bf16:
                                nc.vector.tensor_copy(nxt[p][:, j0:j1, :], tpv)
                            else:
                                nc.vector.tensor_copy(
                                    nxt[:, 2 * p + j0 : 2 * p + j1, :], tpv
                                )

                    if p < 2:
                        emit_tc()
                    else:
                        tails.append(emit_tc)

                def tail():
                    for f in tails:
                        f()
                return tail

            repeat = int(os.environ.get("KERNEL_REPEAT", "1"))

            def phases():
                # ---- phase 1: fp8 steps in hardware loop
                if t_loop > 0:
                    with tc.For_i(0, t_loop, unroll) as iv:
                        pend = None
                        for j in range(unroll):
                            cur, nxt = (h8_a, h8_b) if j % 2 == 0 else (h8_b, h8_a)
                            pend = step(iv, j, "f8", cur, nxt, prev_tail=pend)
                        pend()  # body boundary: emit last step's tail in-body
                pend = None
                # ---- phase 2: fp8 steps unrolled; last one emits bf16 state
                for j in range(t_f8tail):
                    t = t_loop + j
                    cur, nxt = (h8_a, h8_b) if j % 2 == 0 else (h8_b, h8_a)
                    if j == t_f8tail - 1:
                        pend = step(t, 0, "f8", cur, hb_a, out_bf16=True,
                                    prev_tail=pend)
                    else:
                        pend = step(t, 0, "f8", cur, nxt, prev_tail=pend)
                # ---- phase 3: bf16 cleanup steps
                for j in range(t_bf16):
                    t = t_loop + t_f8tail + j
                    cur, nxt = (hb_a, hb_b) if j % 2 == 0 else (hb_b, hb_a)
                    pend = step(t, 0, "bf", cur, nxt, out_bf16=True,
                                prev_tail=pend)
                pend()

            if repeat == 1:
                phases()
            else:  # timing amplification: state re-evolves from same xs
                with tc.For_i(0, repeat, 1):
                    phases()

            ht_fin = hb_a if t_bf16 % 2 == 0 else hb_b

            # final FC: out = h_T @ fc_W.T + fc_b
            fc_ps = psum.tile([B, 2, GW], F32, tag="gates", name="fcps")[:, 0, 0:O]
            nc.tensor.matmul(
                fc_ps, lhsT=ones_t, rhs=fcb_t, start=True, stop=False
            )
            for k in range(KC):
                nc.tensor.matmul(
                    fc_ps,
                    lhsT=ht_fin[:, k],
                    rhs=fcw[:, k * 128 : (k + 1) * 128],
                    start=False,
                    stop=(k == KC - 1),
                )
            out_sb = work.tile([B, O], F32, tag="out_sb")
            nc.vector.tensor_copy(out_sb, fc_ps)
            nc.gpsimd.dma_start(out=out_d, in_=out_sb)

    if split_waits:
        _split_excess_waits(nc)
    return nc


# ------------------------------------------------------------------ host prep
def _prep_inputs(y_hist, W_ih, W_hh, b_ih, b_hh, fc_W, fc_b, h0, c0):
    f32 = np.float32
    bf16 = ml_dtypes.bfloat16
    f8 = ml_dtypes.float8_e4m3
    # per-chunk gate permutation of the 4H rows: [i_c | f_c | o_c | g_c]
    # reference gate order in rows is (i, f, g, o) blocks of H
    perm = np.concatenate(
        [
            g * H + c * 128 + np.arange(128)
            for c in range(KC)
            for g in (0, 1, 3, 2)
        ]
    )
    wt = np.ascontiguousarray(W_hh[perm, :].T).astype(f32)            # (H, 4H)
    w8 = wt.reshape(NP, 2, 128, 4 * H).transpose(2, 0, 1, 3).astype(f8)
    wbf = wt.reshape(KC, 128, 4 * H).transpose(1, 0, 2).astype(bf16)
    wb = np.broadcast_to(W_ih[:, 0][perm].astype(bf16)[None, :], (128, 4 * H))
    bbv = (b_ih + b_hh)[perm].astype(f32)
    bb = np.broadcast_to(bbv[None, :], (128, 4 * H))
    xct = np.ascontiguousarray(y_hist).astype(f32)                    # (B, T)
    xaug = np.stack([y_hist, np.ones((B, T), f32)], axis=0).astype(bf16)
    wba = np.stack([W_ih[:, 0][perm], bbv]).astype(bf16)              # (2, 4H)
    ht0 = np.ascontiguousarray(h0.T).astype(f32)                      # (H, B)
    h80 = ht0.reshape(NP, 2, 128, B).transpose(2, 0, 1, 3).astype(f8)
    fcw = np.ascontiguousarray(fc_W.T).astype(bf16)                   # (H, O)
    fcw_tile = fcw.reshape(KC, 128, O).transpose(1, 0, 2).reshape(128, H)
    fca = np.stack([fc_b, np.ones(O, f32)]).astype(bf16)
    ident = np.eye(128, dtype=f32).astype(bf16)
    return {
        "ident": np.asarray(ident),
        "w8": np.ascontiguousarray(w8),
        "wbf": np.ascontiguousarray(wbf),
        "wb": np.ascontiguousarray(wb),
        "bb": np.ascontiguousarray(bb),
        "xct": xct,
        "xaug": np.ascontiguousarray(xaug),
        "wba": np.ascontiguousarray(wba),
        "h80": np.ascontiguousarray(h80),
        "c0": c0.astype(f32),
        "fcw": np.asarray(fcw_tile),
        "fca": np.asarray(fca),
    }


# ---------------------------------------------------------------- jit runner
_CACHE = {}


def _make_runner(nc):
    import jax
    from concourse import bass2jax

    bass2jax.install_neuronx_cc_hook()
    partition_name = nc.partition_id_tensor.name if nc.partition_id_tensor else None
    in_names, out_names, out_avals, zero_outs = [], [], [], []
    for alloc in nc.m.functions[0].allocations:
        if not isinstance(alloc, mybir.MemoryLocationSet):
            continue
        name = alloc.memorylocations[0].name
        if alloc.kind == "ExternalInput":
            if name != partition_name:
                in_names.append(name)
        elif alloc.kind == "ExternalOutput":
            shape = tuple(alloc.tensor_shape)
            dtype = mybir.dt.np(alloc.dtype)
            out_names.append(name)
            out_avals.append(jax.core.ShapedArray(shape, dtype))
            zero_outs.append(np.zeros(shape, dtype))
    all_in = list(in_names) + list(out_names)
    if partition_name is not None:
        all_in.append(partition_name)

    def _body(*args):
        operands = list(args)
        if partition_name is not None:
            operands.append(bass2jax.partition_id_tensor())
        return tuple(
            bass2jax._bass_exec_p.bind(
                *operands,
                out_avals=tuple(out_avals),
                in_names=tuple(all_in),
                out_names=tuple(out_names),
                lowering_input_output_aliases=(),
                sim_require_finite=True,
                sim_require_nnan=True,
                nc=nc,
            )
        )

    f = jax.jit(_body, keep_unused=True)
    return f, in_names, out_names, zero_outs


def kernel(y_hist, W_ih, W_hh, b_ih, b_hh, fc_W, fc_b, h0, c0, **kw):
    dev_in = _prep_inputs(
        np.asarray(y_hist, np.float32),
        np.asarray(W_ih, np.float32),
        np.asarray(W_hh, np.float32),
        np.asarray(b_ih, np.float32),
        np.asarray(b_hh, np.float32),
        np.asarray(fc_W, np.float32),
        np.asarray(fc_b, np.float32),
        np.asarray(h0, np.float32),
        np.asarray(c0, np.float32),
    )
    if "runner" not in _CACHE:
        nc = build_program()
        _CACHE["runner"] = _make_runner(nc)
    f, in_names, out_names, zero_outs = _CACHE["runner"]
    args = [np.asarray(dev_in[n]) for n in in_names] + list(zero_outs)
    outs = f(*args)
    res = {n: np.asarray(outs[i]) for i, n in enumerate(out_names)}
    return np.asarray(res["out"], np.float32)
